# revision 74
# baseline (speedup 1.0000x reference)
"""MetapathAttentionLayer Trainium2 kernel.

Math (per node n):
    scores[n, m] = sum_d x[m, n, d] * W[d, m]
    att = softmax(relu(scores), axis=m)      (8 metapaths)
    out[n, :] = elu(sum_m att[n, m] * x[m, n, :])

Strategy: shard nodes across 8 cores (data parallel). Per core, x lives
in node-major layout [nodes(part), d(free)] in bf16; T-tiles of 14
chunks (98 = 7x14, no ragged tail), software-pipelined with a 2-tile
lookahead between the load/score phase and the pooling phase.

  - scores, hybrid split across engines to unload the DVE:
      m < K_PE=3: the same slabs are ALSO loaded d-major via an XBAR
        dma_start_transpose (host pre-interleaves them per T-tile so one
        transpose instruction covers all 3), scores = PE matmuls with
        masked-W stationaries accumulating rows of S^T [3, nt] in PSUM,
        ACT-copied to SBUF bf16, then re-blocked to node-major chunks by
        tiny PE matmuls (lhsT = strided S^T slice, rhs = I16 identity:
        out[p, m] = S^T[m, p*ct + c], ~7ns each)
      m >= K_PE: DVE tensor_tensor vs replicated-W + 3 bf16 tree-fold
        adds (2x mode) + one tensor_reduce
  - softmax: both score pieces exp'd (ACT) straight into one (c m)-major
    e tile; e = exp(relu(s)) = max(exp(s), 1) via DVE max
  - pooling: PE matmuls with diag(e_m) stationary, accumulating over m
    into PSUM; diag built by GPSIMD local_scatter (3 of every 4 chunks)
    or DVE tensor_scalar (every 4th) to balance the engines.  diag uses
    UNNORMALIZED e: the softmax 1/sum rides into the elu as a
    per-partition ACT scale, keeping recip/att off the scatter path.
  - elu(y)=max(y, exp(min(y,0))-1) with y = inv*x: 3 per-chunk ACT ops
    (Copy-scale, Relu, Exp) + one DVE scalar_tensor_tensor combine
  - bf16 output DMA (host upconverts), issued per 4-chunk group so the
    store drains as soon as each group's elu lands.  DMA granularity
    tuned for the
    serialized DMA pool; issue order per T-tile: the m >= K_PE halves of
    the node-major load first (they alone gate the DVE product), then
    the XBAR transpose in four 448-node pieces (so dependent score
    matmuls start at quarter-latency), then the pooling-only m < K_PE
    slab.
"""

import os
from contextlib import ExitStack

import numpy as np
import ml_dtypes

import concourse.bass as bass
import concourse.tile as tile
from concourse import bacc, mybir, library_config
import concourse.bass_utils as bass_utils

F32 = mybir.dt.float32
BF16 = mybir.dt.bfloat16
I16 = mybir.dt.int16
ALU = mybir.AluOpType
ACTF = mybir.ActivationFunctionType

NMETA = 8
N = 100000
D = 128
NCORES = 8
NC_RAW = N // NCORES          # 12500 nodes per core
CHUNK = 128                   # nodes per compute chunk (partition dim)
NC_PAD = 12544                # 98 chunks of 128
T_CHUNKS = 14                 # chunks per DMA T-tile (1024 nodes)
GROUP = 4                     # chunks per PSUM/elu group (psum bank = 512 f32)

# tunables
DIAG_DVE_EVERY = 4   # every k-th chunk builds diag via DVE tensor_scalar (0=off)
K_PE = 3             # metapaths whose scores run on the PE (d-major path)


def kernel_body(tc, out_d, x_d, wb_d, sidx_d, icat_d, wm_d, xt_d,
                nc_pad=NC_PAD, t_chunks=T_CHUNKS, reps=1,
                diag_dve_every=DIAG_DVE_EVERY, k_pe=K_PE, comb_on_pool=False):
    nc = tc.nc
    md = NMETA - k_pe             # metapaths on the DVE score path
    with ExitStack() as ctx:
        const = ctx.enter_context(tc.tile_pool(name="const", bufs=1))
        xpool = ctx.enter_context(tc.tile_pool(name="x", bufs=3))
        xtpool = ctx.enter_context(tc.tile_pool(name="xt", bufs=3))
        stpool = ctx.enter_context(tc.tile_pool(name="st", bufs=3))
        opool = ctx.enter_context(tc.tile_pool(name="o", bufs=2))
        ppool = ctx.enter_context(tc.tile_pool(name="prod", bufs=_pb))
        fpool = ctx.enter_context(tc.tile_pool(name="fold", bufs=_pb))
        spool = ctx.enter_context(tc.tile_pool(name="smalls", bufs=6))
        dpool = ctx.enter_context(tc.tile_pool(name="diag", bufs=_db))
        epool = ctx.enter_context(tc.tile_pool(name="elu", bufs=_pb))
        psum = ctx.enter_context(tc.tile_pool(name="ps", bufs=4, space="PSUM"))
        sscp = ctx.enter_context(tc.tile_pool(name="ssc", bufs=ssc_bufs, space="PSUM"))
        pscp = ctx.enter_context(tc.tile_pool(name="psc", bufs=psc_bufs, space="PSUM"))

        wb = const.tile([128, NMETA * D], BF16)
        nc.sync.dma_start(wb[:], wb_d[:])
        sidx = const.tile([128, NMETA], I16)
        nc.sync.dma_start(sidx[:], sidx_d[:])
        icat = const.tile([128, NMETA * D], BF16)
        nc.sync.dma_start(icat[:], icat_d[:])
        wmat = const.tile([128, NMETA * NMETA], BF16)
        nc.sync.dma_start(wmat[:], wm_d[:])
        nc.gpsimd.load_library(library_config.local_scatter)

        chunk_idx = 0
        for _rep in range(reps):
            cts = []
            rem = nc_pad // CHUNK
            while rem > 0:
                ct = min(t_chunks, rem)
                cts.append(ct)
                rem -= ct
            if taper_end and cts[-1] == t_chunks:
                cts[-1:] = [t_chunks // 2, t_chunks - t_chunks // 2]
            tiles = []
            n0 = 0
            for ct in cts:
                tiles.append((n0, ct))
                n0 += ct * CHUNK

            def phase_a(n0, ct):
                """Loads + PE score path for one T-tile (no DVE work)."""
                nt = ct * CHUNK
                # node n = n0 + p*ct + c  ->  partition p, free chunk c.
                # Two DMAs: the m >= k_pe slab first (it alone gates the
                # DVE product), the pooling-only m < k_pe slab second.
                # two separate tiles so the DVE product (m >= k_pe only)
                # never waits on the pooling-only m < k_pe transfer
                Xg = xpool.tile([128, md * nt], BF16, tag="Xg")
                Xl = xpool.tile([128, k_pe * nt], BF16, tag="Xl")
                Xgw = Xg[:].rearrange("p (m c d) -> p m c d", m=md, c=ct)
                Xlw = Xl[:].rearrange("p (m c d) -> p m c d", m=k_pe, c=ct)
                xsrc = x_d[:, n0:n0 + nt, :].rearrange(
                    "m (p c) d -> p m c d", p=128)
                mh = (NMETA + k_pe) // 2
                def _x_mge():
                    nc.sync.dma_start(Xgw[:, :mh - k_pe], xsrc[:, k_pe:mh])
                    nc.sync.dma_start(Xgw[:, mh - k_pe:], xsrc[:, mh:])
                def _x_mlt():
                    nc.sync.dma_start(Xlw[:], xsrc[:, :k_pe])
                if dma_order == 0:
                    _x_mge(); _x_mlt()
                ps_sc = None
                if k_pe:
                    # PE score path for m < k_pe: XBAR-transposed load of
                    # the same slabs as [d(part), node(free)], scores via
                    # masked-W matmuls into S^T psum rows, bf16 copy to
                    # SBUF, then re-block to node-major via tiny identity
                    # matmuls: out[p, m] = S^T[m, p*ct + c]
                    # XT layout [128, (half, m, nh)]: the host interleaves
                    # xt at half-tile granularity so each half is one
                    # contiguous 2D transpose (finer DMA-pool granularity).
                    XT = xtpool.tile([128, k_pe * nt], BF16, tag="XT")
                    nh = 448
                    if dma_order == 1:
                        _x_mge()
                    for h in range(nt // nh):
                        nc.sync.dma_start_transpose(
                            XT[:, h * k_pe * nh:(h + 1) * k_pe * nh],
                            xt_d[k_pe * (n0 + h * nh):
                                 k_pe * (n0 + (h + 1) * nh), :])
                    if dma_order == 1:
                        _x_mlt()
                    elif dma_order == 2:
                        _x_mge(); _x_mlt()
                    STb = stpool.tile([k_pe, nt], BF16, tag="STb")
                    for h in range(nt // nh):
                        for b0 in range(0, nh, 512):
                            bl = min(512, nh - b0)
                            ssc = sscp.tile([k_pe, 512], F32, tag="ssc")
                            for m in range(k_pe):
                                nc.tensor.matmul(
                                    out=ssc[:, :bl],
                                    lhsT=wmat[:].rearrange(
                                        "p (m k) -> p m k",
                                        m=NMETA)[:, m, 0:k_pe],
                                    rhs=XT[:, h * k_pe * nh + m * nh + b0:
                                           h * k_pe * nh + m * nh + b0 + bl],
                                    start=(m == 0), stop=(m == k_pe - 1))
                            nc.scalar.activation(
                                STb[:, h * nh + b0:h * nh + b0 + bl],
                                ssc[:, :bl], ACTF.Copy)
                    STv = STb[:].rearrange("q (p c) -> q c p", c=ct)
                    ps_sc = pscp.tile([128, t_chunks * 16], F32, tag="psc")
                    for c in range(ct):
                        nc.tensor.matmul(
                            out=ps_sc[:, c * 16:(c + 1) * 16],
                            lhsT=STv[:, c, :],
                            rhs=icat[0:k_pe, 0:16],
                            start=True, stop=True)
                return Xg, Xl, ps_sc

            def phase_b(n0, ct, Xg, Xl, ps_sc, is_last=False):
                nonlocal chunk_idx
                nt = ct * CHUNK
                Xgv = Xg[:].rearrange("p (m c d) -> p m c d", m=md, c=ct)
                Xlv = Xl[:].rearrange("p (m c d) -> p m c d", m=k_pe, c=ct)
                out_sb = opool.tile([128, nt], BF16, tag="osb")
                for g0 in range(0, ct, GROUP):
                    gl = min(GROUP, ct - g0)
                    mc = md * gl
                    ps = psum.tile([128, GROUP * D], F32, tag="ps")

                    # DVE score path for m >= k_pe
                    P = ppool.tile([128, NMETA * GROUP * D], BF16, tag="P")
                    Pv = P[:, :md * GROUP * D].rearrange(
                        "p (m c d) -> p m c d", m=md, c=GROUP)
                    nc.vector.tensor_tensor(
                        out=Pv[:, :, :gl, :],
                        in0=Xgv[:, :, g0:g0 + gl, :],
                        in1=wb[:].rearrange("p (m d) -> p m d", m=NMETA)
                              [:, k_pe:, :]
                              .unsqueeze(2).broadcast_to([128, md, gl, D]),
                        op=ALU.mult,
                    )
                    # scores[p, (m c)] = sum_d P: fold d 128->16 in bf16 (2x
                    # DVE), then one 1x tensor_reduce.
                    Pg = Pv[:, :, :gl, :]
                    f1 = fpool.tile([128, NMETA * GROUP * D // 2], BF16, tag="f1")
                    f1v = f1[:, :md * GROUP * D // 2].rearrange(
                        "p (m c d) -> p m c d", m=md, c=GROUP)[:, :, :gl, :]
                    nc.vector.tensor_tensor(
                        out=f1v, in0=Pg[:, :, :, 0:64], in1=Pg[:, :, :, 64:128],
                        op=ALU.add)
                    f2 = fpool.tile([128, NMETA * GROUP * D // 4], BF16, tag="f2")
                    f2v = f2[:, :md * GROUP * D // 4].rearrange(
                        "p (m c d) -> p m c d", m=md, c=GROUP)[:, :, :gl, :]
                    nc.vector.tensor_tensor(
                        out=f2v, in0=f1v[:, :, :, 0:32], in1=f1v[:, :, :, 32:64],
                        op=ALU.add)
                    f3 = fpool.tile([128, NMETA * GROUP * D // 8], BF16, tag="f3")
                    f3v = f3[:, :md * GROUP * D // 8].rearrange(
                        "p (m c d) -> p m c d", m=md, c=GROUP)[:, :, :gl, :]
                    nc.vector.tensor_tensor(
                        out=f3v, in0=f2v[:, :, :, 0:16], in1=f2v[:, :, :, 16:32],
                        op=ALU.add)
                    scores = spool.tile([128, GROUP * NMETA], F32, tag="scores")
                    nc.vector.tensor_reduce(
                        out=scores[:, :mc].rearrange("p (m c) -> p m c", m=md),
                        in_=f3v,
                        axis=mybir.AxisListType.X, op=ALU.add)

                    # softmax over m: merge both score pieces into one
                    # (c m)-major e tile: e = exp(relu(s)) = max(exp(s), 1)
                    e_bf = spool.tile([128, GROUP * NMETA], BF16, tag="ebf")
                    e_cm = e_bf[:, :gl * NMETA].rearrange(
                        "p (c m) -> p c m", m=NMETA)
                    if k_pe:
                        nc.scalar.activation(
                            e_cm[:, :, 0:k_pe],
                            ps_sc[:, g0 * 16:(g0 + gl) * 16].rearrange(
                                "p (c s) -> p c s", s=16)[:, :, 0:k_pe],
                            ACTF.Exp)
                    nc.scalar.activation(
                        e_cm[:, :, k_pe:].rearrange("p c m -> p m c"),
                        scores[:, :mc].rearrange("p (m c) -> p m c", m=md),
                        ACTF.Exp)
                    e_max = spool.tile([128, GROUP * NMETA], BF16, tag="emax")
                    nc.vector.tensor_scalar(
                        e_max[:, :gl * NMETA], e_bf[:, :gl * NMETA],
                        1.0, None, ALU.max)
                    em_cm = e_max[:, :gl * NMETA].rearrange(
                        "p (c m) -> p c m", m=NMETA)
                    sums = spool.tile([128, GROUP], F32, tag="sums")
                    nc.vector.tensor_reduce(
                        out=sums[:, :gl], in_=em_cm,
                        axis=mybir.AxisListType.X,
                        op=ALU.add,
                    )
                    inv = spool.tile([128, GROUP], F32, tag="inv")
                    nc.vector.reciprocal(inv[:, :gl], sums[:, :gl])
                    att_g = spool.tile([128, GROUP * NMETA], BF16, tag="attg")
                    ag_cm = att_g[:, :gl * NMETA].rearrange(
                        "p (c m) -> p c m", m=NMETA)
                    nc.vector.tensor_tensor(
                        out=ag_cm, in0=em_cm,
                        in1=inv[:, :gl].unsqueeze(2).broadcast_to(
                            [128, gl, NMETA]),
                        op=ALU.mult)
                    att_gf = spool.tile([128, GROUP * NMETA], F32, tag="attgf")
                    agf_cm = att_gf[:, :gl * NMETA].rearrange(
                        "p (c m) -> p c m", m=NMETA)
                    nc.vector.tensor_tensor(
                        out=agf_cm, in0=em_cm,
                        in1=inv[:, :gl].unsqueeze(2).broadcast_to(
                            [128, gl, NMETA]),
                        op=ALU.mult)

                    for cg in range(gl):
                        c = g0 + cg
                        diag = dpool.tile([128, NMETA * D], BF16, tag="diag")
                        use_dve = (diag_dve_every and
                                   chunk_idx % diag_dve_every == dve_ph)
                        if is_last and last_mode == 1:
                            use_dve = chunk_idx % 2 == 0
                        elif is_last and last_mode == 2:
                            use_dve = chunk_idx % 2 != 0
                        if use_dve:
                            for m in range(NMETA):
                                nc.vector.tensor_scalar(
                                    diag[:, m * D:(m + 1) * D],
                                    icat[:, m * D:(m + 1) * D],
                                    ef_cm[:, cg, m:m + 1], None, ALU.mult)
                        else:
                            nc.gpsimd.local_scatter(
                                diag[:], e_max[:, cg * NMETA:(cg + 1) * NMETA],
                                sidx[:],
                                channels=128, num_elems=NMETA * D,
                                num_idxs=NMETA)
                        for m in range(NMETA):
                            nc.tensor.matmul(
                                out=ps[:, cg * D:(cg + 1) * D],
                                lhsT=diag[:, m * D:(m + 1) * D],
                                rhs=(Xlv[:, m, c, :] if m < k_pe
                                     else Xgv[:, m - k_pe, c, :]),
                                start=(m == 0),
                                stop=(m == NMETA - 1),
                            )
                        chunk_idx += 1

                    # elu(x) = max(x, exp(min(x,0)) - 1)
                    w = gl * D
                    t = epool.tile([128, GROUP * D], F32, tag="t")
                    nc.scalar.activation(t[:, :w], ps[:, :w], ACTF.Relu,
                                         scale=-1.0)
                    e2 = epool.tile([128, GROUP * D], F32, tag="e2")
                    nc.scalar.activation(e2[:, :w], t[:, :w], ACTF.Exp,
                                         scale=-1.0)
                    # out = max(ps, e2 - 1) in one fused op
                    eng = nc.gpsimd if comb_on_pool else nc.vector
                    eng.scalar_tensor_tensor(
                        out=out_sb[:, g0 * D:g0 * D + w],
                        in0=e2[:, :w], scalar=-1.0, in1=ps[:, :w],
                        op0=ALU.add, op1=ALU.max)

                    # per-group output DMA: drains earlier, releases
                    # out_sb sooner
                    dsto = out_d[n0:n0 + nt, :].rearrange(
                        "(p c) d -> p (c d)", p=128)
                    nc.sync.dma_start(
                        dsto[:, g0 * D:g0 * D + gl * D],
                        out_sb[:, g0 * D:g0 * D + gl * D])

            # software pipeline with one-tile lookahead: tile t+1's loads
            # and PE score phase are emitted before tile t's pooling phase,
            # so the in-order PE queue never parks score matmuls behind
            # diag-gated pooling matmuls.
            pend = []
            for tn0, tct in tiles:
                pend.append((tn0, tct) + phase_a(tn0, tct))
                if len(pend) > lookahead:
                    phase_b(*pend.pop(0))
            for i, pb in enumerate(pend):
                phase_b(*pb, is_last=(i == len(pend) - 1))


def host_inputs(x_np, w_np, nc_pad=NC_PAD):
    """Build per-core input maps from full fp32 inputs."""
    in_maps = []
    wbig = np.ascontiguousarray(
        np.broadcast_to(w_np.T.reshape(1, NMETA * D), (128, NMETA * D))
    ).astype(ml_dtypes.bfloat16)
    sidx = (np.arange(NMETA)[None, :] * D + np.arange(128)[:, None]).astype(np.int16)
    icat = np.ascontiguousarray(
        np.tile(np.eye(128, dtype=np.float32), (1, NMETA))
    ).astype(ml_dtypes.bfloat16)
    # masked-W blocks: wmk[:, m*NMETA + m'] = W[:, m] iff m' == m else 0
    wmk_f = np.zeros((128, NMETA * NMETA), dtype=np.float32)
    for m in range(NMETA):
        wmk_f[:, m * NMETA + m] = w_np[:, m]
    wmk = wmk_f.astype(ml_dtypes.bfloat16)
    nc_raw = x_np.shape[1] // NCORES
    for c in range(NCORES):
        xs = x_np[:, c * nc_raw:(c + 1) * nc_raw, :]
        xp = np.zeros((NMETA, nc_pad, D), dtype=ml_dtypes.bfloat16)
        xp[:, :nc_raw, :] = xs.astype(ml_dtypes.bfloat16)
        blocks = []
        n0 = 0
        while n0 < nc_pad:
            nt = min(448, nc_pad - n0)
            blocks.append(
                np.ascontiguousarray(xp[:K_PE, n0:n0 + nt, :])
                .reshape(K_PE * nt, D))
            n0 += nt
        xtl = np.concatenate(blocks, axis=0)
        in_maps.append({"x": xp, "wb": wbig, "sidx": sidx, "icat": icat,
                        "wm": wmk, "xt": xtl})
    return in_maps


_CACHE = {}


def build(reps=1, **kw):
    key = (reps, tuple(sorted(kw.items())))
    if key in _CACHE:
        return _CACHE[key]
    nc = bacc.Bacc("TRN2", target_bir_lowering=False, debug=False,
                   num_devices=NCORES)
    x = nc.dram_tensor("x", [NMETA, NC_PAD, D], BF16, kind="ExternalInput").ap()
    wb = nc.dram_tensor("wb", [128, NMETA * D], BF16, kind="ExternalInput").ap()
    sidx = nc.dram_tensor("sidx", [128, NMETA], I16, kind="ExternalInput").ap()
    icat = nc.dram_tensor("icat", [128, NMETA * D], BF16, kind="ExternalInput").ap()
    wm = nc.dram_tensor("wm", [128, NMETA * NMETA], BF16,
                        kind="ExternalInput").ap()
    xt = nc.dram_tensor("xt", [K_PE * NC_PAD, D], BF16,
                        kind="ExternalInput").ap()
    out = nc.dram_tensor("out", [NC_PAD, D], BF16, kind="ExternalOutput").ap()
    with tile.TileContext(nc) as tc:
        kernel_body(tc, out, x, wb, sidx, icat, wm, xt, reps=reps, **kw)
    nc.compile()
    _CACHE[key] = nc
    return nc


def run(input, W, trace=False, **trace_kwargs):
    x_np = np.asarray(input, dtype=np.float32)
    w_np = np.asarray(W, dtype=np.float32)
    nc = build()
    in_maps = host_inputs(x_np, w_np)
    res = bass_utils.run_bass_kernel_spmd(
        nc, in_maps, core_ids=list(range(NCORES)), trace=trace, **trace_kwargs)
    nc_raw = x_np.shape[1] // NCORES
    full = np.concatenate(
        [np.asarray(res.results[c]["out"][:nc_raw], dtype=np.float32)
         for c in range(NCORES)], axis=0)
    return full, res


def kernel(input, W):
    out, _ = run(input, W, trace=False)
    return out


# ---------------------------------------------------------------------------
# Timing harness (test-only): persistent jit over the bass_exec primitive so
# repeated executions reuse device-resident inputs. HW kernel time is derived
# from the slope between an R-repeat NEFF and the 1-repeat NEFF.
# ---------------------------------------------------------------------------

def make_runner(nc):
    import jax
    from jax.experimental.shard_map import shard_map
    from jax.sharding import Mesh, PartitionSpec, NamedSharding
    from concourse import bass2jax as b2j

    b2j.install_neuronx_cc_hook()
    partition_name = nc.partition_id_tensor.name if nc.partition_id_tensor else None
    in_names, out_names, out_avals, zero_outs = [], [], [], []
    for alloc in nc.m.functions[0].allocations:
        if not isinstance(alloc, mybir.MemoryLocationSet):
            continue
        name = alloc.memorylocations[0].name
        if alloc.kind == "ExternalInput":
            if name != partition_name:
                in_names.append(name)
        elif alloc.kind == "ExternalOutput":
            out_names.append(name)
            shape = tuple(alloc.tensor_shape)
            dtype = mybir.dt.np(alloc.dtype)
            out_avals.append(jax.core.ShapedArray(shape, dtype))
            zero_outs.append(np.zeros(shape, dtype))
    n_params = len(in_names)
    n_outs = len(out_avals)
    all_names = in_names + out_names + ([partition_name] if partition_name else [])

    def _body(*args):
        operands = list(args)
        if partition_name is not None:
            operands.append(b2j.partition_id_tensor())
        outs = b2j._bass_exec_p.bind(
            *operands,
            out_avals=tuple(out_avals),
            in_names=tuple(all_names),
            out_names=tuple(out_names),
            lowering_input_output_aliases=(),
            sim_require_finite=True,
            sim_require_nnan=True,
            nc=nc,
        )
        return tuple(outs)

    devices = jax.devices()[:NCORES]
    mesh = Mesh(np.asarray(devices), ("core",))
    in_specs = (PartitionSpec("core"),) * (n_params + n_outs)
    out_specs = (PartitionSpec("core"),) * n_outs
    donate = tuple(range(n_params, n_params + n_outs))
    sharded = jax.jit(
        shard_map(_body, mesh=mesh, in_specs=in_specs, out_specs=out_specs,
                  check_rep=False),
        donate_argnums=donate, keep_unused=True)
    sharding = NamedSharding(mesh, PartitionSpec("core"))
    return sharded, in_names, zero_outs, sharding


class _TimedRunner:
    def __init__(self, nc, in_maps):
        import jax
        self.jax = jax
        sharded, in_names, zero_outs, sharding = make_runner(nc)
        self.sharded = sharded
        concat_in = [
            np.concatenate([in_maps[c][n] for c in range(NCORES)], axis=0)
            for n in in_names
        ]
        self.xs = [jax.device_put(a, sharding) for a in concat_in]
        self.zero_outs = zero_outs
        self.sharding = sharding

    def _zset(self):
        return [
            self.jax.device_put(
                np.zeros((NCORES * z.shape[0], *z.shape[1:]), z.dtype),
                self.sharding)
            for z in self.zero_outs
        ]

    def piped(self, reps):
        import time as _t
        zsets = [self._zset() for _ in range(reps + 1)]
        self.jax.block_until_ready(zsets)
        self.jax.block_until_ready(self.xs)
        o = self.sharded(*self.xs, *zsets[0])
        self.jax.block_until_ready(o)
        _ = self.jax.device_get(o[0])
        t0 = _t.perf_counter()
        outs = [self.sharded(*self.xs, *zsets[1 + k]) for k in range(reps)]
        self.jax.block_until_ready(outs)
        # force true device completion: fetch the last output's bytes
        _ = self.jax.device_get(outs[-1][0])
        return (_t.perf_counter() - t0) / reps


def measure(input, W, reps=12, neff_reps=9, rounds=4, **kw):
    """Estimate per-iteration HW time via multi-repeat NEFF slope.

    Interleaves rounds of (1-repeat NEFF, R-repeat NEFF) piped timings and
    takes the min across rounds for each to reject dispatch-overhead noise.
    """
    x_np = np.asarray(input, dtype=np.float32)
    w_np = np.asarray(W, dtype=np.float32)
    in_maps = host_inputs(x_np, w_np)

    nc1 = build(reps=1, **kw)
    ncr = build(reps=neff_reps, **kw)
    r1 = _TimedRunner(nc1, in_maps)
    rr = _TimedRunner(ncr, in_maps)
    t1s, trs = [], []
    for _ in range(rounds):
        t1s.append(r1.piped(reps))
        trs.append(rr.piped(reps))
    t1, tr = min(t1s), min(trs)
    slope = (tr - t1) / (neff_reps - 1)
    return t1, tr, slope, t1s, trs

